# revision 1
# baseline (speedup 1.0000x reference)
"""Trainium2 Bass kernel for GQA sliding-window attention (nn_Attention_20375324852422).

Reference computation (B=2, T=2048, D=2560, N=8 q-heads, K=4 kv-heads, H=256,
WINDOW=1024):
    q = x @ q_w ; k,v = x @ kv_w      (GQA projections)
    q,k = rms_norm(q/k, scale)        (QK-norm, (1+scale) gain)
    q,k = rope(q/k, positions); q *= H**-0.5
    logits = q @ k.T  (grouped, sliding-window causal mask), softmax
    out = (probs @ v) @ out_w

Sharding: 8 cores = 2 (batch) x 4 (sequence chunks of 512 queries).  Each core
recomputes k/v for its 1536-key window (chunks j-2..j, zero-padded below 0), so
there are ZERO collectives - every core is fully independent.  Weights are
replicated, activations bf16, accumulation fp32.

Per-core device layouts (host prepares, see kernel()):
    xT   [D, 1536]   x window transposed (bf16)  - lhsT for projections
    qw   [D, N*H]    kw/vw [D, K*H]              - rhs for projections
    ow   [N*H, D]                                - rhs for output projection
    tq   [512, 4, 128] / tk [1536, 4, 128]       - RoPE tables (C1,S1,C2,S2)
                     with (1+scale) gains and (for q) H**-0.5 folded in
    m01  [1536, 512] multiplicative {0,1} mask, transposed (keys x queries)
    out  [512, D]    fp32
"""

import numpy as np
import ml_dtypes

import concourse.bass as bass
import concourse.tile as tile
from concourse import bacc
from concourse import mybir
from concourse.masks import make_identity

BF16 = mybir.dt.bfloat16
F32 = mybir.dt.float32

B, T, D, N, K, H = 2, 2048, 2560, 8, 4, 256
G = N // K
WINDOW = 1024
ROPE_BASE = 10000
EPS = 1e-6
HH = H // 2  # 128


def build_nc(d_tiles=20, sw_tiles=12, sq_tiles=4, nh=8, kh=4):
    """Build the per-core Bass graph. Sizes in units of 128 (partition tiles)."""
    nc = bacc.Bacc()
    d = d_tiles * 128
    sw = sw_tiles * 128
    sq = sq_tiles * 128
    g = nh // kh

    xT_e = nc.dram_tensor("xT", [d, sw], BF16, kind="ExternalInput")
    qw_e = nc.dram_tensor("qw", [d, nh * H], BF16, kind="ExternalInput")
    kw_e = nc.dram_tensor("kw", [d, kh * H], BF16, kind="ExternalInput")
    vw_e = nc.dram_tensor("vw", [d, kh * H], BF16, kind="ExternalInput")
    ow_e = nc.dram_tensor("ow", [nh * H, d], BF16, kind="ExternalInput")
    tq_e = nc.dram_tensor("tq", [sq, 4, HH], BF16, kind="ExternalInput")
    tk_e = nc.dram_tensor("tk", [sw, 4, HH], BF16, kind="ExternalInput")
    m01_e = nc.dram_tensor("m01", [sw, sq], BF16, kind="ExternalInput")
    out_e = nc.dram_tensor("out", [sq, d], F32, kind="ExternalOutput")

    # free-dim chunking of the projection rhs (<=512 per matmul = one PSUM bank)
    qch_f = min(512, nh * H)
    QCH = (nh * H) // qch_f
    hpq = qch_f // H                  # heads per q chunk
    kch_f = min(512, kh * H)
    KCH = (kh * H) // kch_f
    hpk = kch_f // H

    with tile.TileContext(nc) as tc:
        with (
            tc.tile_pool(name="const", bufs=1) as const,
            tc.tile_pool(name="persist", bufs=1) as persist,
            tc.tile_pool(name="psA", bufs=3, space="PSUM") as psA,
            tc.tile_pool(name="psT", bufs=2, space="PSUM") as psT,
        ):
            ident = const.tile([128, 128], BF16)
            make_identity(nc, ident)
            eps_t = const.tile([128, 1], F32)
            nc.vector.memset(eps_t, EPS)

            kT = persist.tile([128, kh * 2, sw], BF16)      # [h, kv-head*half, s]
            v_sb = persist.tile([128, sw_tiles, kh, H + 1], BF16)
            qT = persist.tile([128, nh * 2, sq], BF16)
            encT = persist.tile([128, nh * 2, sq], BF16)
            tq_sb = persist.tile([128, sq_tiles, 4, HH], BF16)
            tk_sb = persist.tile([128, sw_tiles, 4, HH], BF16)


            def rope(dst, src, tbl, heads):
                """dst/src: [128, heads, H] bf16 sbuf; tbl: [128, 4, HH] slice."""
                first = src[:, :, 0:HH]
                second = src[:, :, HH:H]
                c1 = tbl[:, 0, :].unsqueeze(1).broadcast_to([128, heads, HH])
                s1 = tbl[:, 1, :].unsqueeze(1).broadcast_to([128, heads, HH])
                c2 = tbl[:, 2, :].unsqueeze(1).broadcast_to([128, heads, HH])
                s2 = tbl[:, 3, :].unsqueeze(1).broadcast_to([128, heads, HH])
                t1 = scratch.tile([128, heads, HH], F32, tag="rp1", name="t1")
                t2 = scratch.tile([128, heads, HH], F32, tag="rp2", name="t2")
                nc.vector.tensor_mul(t1, first, c1)
                nc.vector.tensor_mul(t2, second, s1)
                nc.vector.tensor_sub(dst[:, :, 0:HH], t1, t2)
                nc.vector.tensor_mul(t1, second, c2)
                nc.vector.tensor_mul(t2, first, s2)
                nc.vector.tensor_add(dst[:, :, HH:H], t1, t2)

            def norm_scale_copy(dst, psrc, heads):
                """RMS-normalize psum [128, heads*H] into sbuf dst [128, heads, H]."""
                ssq = scratch.tile([128, heads], F32, tag="ssq", name="ssq")
                sq_junk = scratch.tile([128, H], BF16, tag="sqj", name="sqj")
                for hh in range(heads):
                    nc.scalar.activation(
                        out=sq_junk, in_=psrc[:, hh * H:(hh + 1) * H],
                        func=mybir.ActivationFunctionType.Square,
                        accum_out=ssq[:, hh:hh + 1])
                nc.scalar.activation(
                    out=ssq, in_=ssq, func=mybir.ActivationFunctionType.Sqrt,
                    bias=eps_t, scale=1.0 / H)
                nc.vector.reciprocal(ssq, ssq)
                for hh in range(heads):
                    nc.scalar.activation(
                        out=dst[:, hh, :], in_=psrc[:, hh * H:(hh + 1) * H],
                        func=mybir.ActivationFunctionType.Copy,
                        scale=ssq[:, hh:hh + 1])

            with (
                tc.tile_pool(name="xpool", bufs=1) as xpool,
                tc.tile_pool(name="wstream", bufs=2) as wstream,
                tc.tile_pool(name="scratch", bufs=2) as scratch,
            ):
                # xT loaded in 512-column chunks, first chunk interleaved with
                # the first weight chunk so the first PSUM group isn't queued
                # behind the whole 8MB xT load.
                SC = min(512, sw)
                n_sc = sw // SC
                xT_sb = []
                for dt in range(d_tiles):
                    xt = xpool.tile([128, sw], BF16, tag=f"x{dt}", name=f"x{dt}")
                    xT_sb.append(xt)

                def load_x_chunk(sc):
                    for dt in range(d_tiles):
                        nc.sync.dma_start(
                            out=xT_sb[dt][:, sc * SC:(sc + 1) * SC],
                            in_=xT_e[dt * 128:(dt + 1) * 128, sc * SC:(sc + 1) * SC])

                # rope tables are tiny but gate the DVE rope pipeline: load
                # the k tables before anything else.
                nc.sync.dma_start(
                    out=tk_sb, in_=tk_e.rearrange("(t p) f h -> p t f h", p=128))

                # ---- K/V projection over the full window ----
                # chunk-outer: stream each 512-wide weight chunk once, reuse
                # across all window s-tiles.
                for c in range(KCH):
                    kwc = []
                    for dt in range(d_tiles):
                        kwt = wstream.tile([128, kch_f], BF16, tag=f"w{dt}", bufs=2,
                                           name=f"kw{dt}")
                        nc.sync.dma_start(
                            out=kwt,
                            in_=kw_e[dt * 128:(dt + 1) * 128, c * kch_f:(c + 1) * kch_f])
                        kwc.append(kwt)
                        if c == 0:
                            nc.sync.dma_start(
                                out=xT_sb[dt][:, 0:SC],
                                in_=xT_e[dt * 128:(dt + 1) * 128, 0:SC])
                    if c == 0:
                        nc.sync.dma_start(
                            out=tq_sb, in_=tq_e.rearrange("(t p) f h -> p t f h", p=128))
                        for dt in range(d_tiles):
                            if sw > SC:
                                nc.sync.dma_start(
                                    out=xT_sb[dt][:, SC:sw],
                                    in_=xT_e[dt * 128:(dt + 1) * 128, SC:sw])
                    for st in range(sw_tiles):
                        pk = psA.tile([128, kch_f], F32, tag="pa0", bufs=4, name="pk")
                        for dt in range(d_tiles):
                            nc.tensor.matmul(pk, xT_sb[dt][:, st * 128:(st + 1) * 128],
                                             kwc[dt],
                                             start=(dt == 0), stop=(dt == d_tiles - 1))
                        k_n = scratch.tile([128, hpk, H], BF16, tag="x_n", bufs=3, name="k_n")
                        k_r = scratch.tile([128, hpk, H], BF16, tag="x_r", bufs=3, name="k_r")
                        norm_scale_copy(k_n, pk, hpk)
                        rope(k_r, k_n, tk_sb[:, st, :, :], hpk)
                        for hh in range(hpk):
                            for half in range(2):
                                pt = psT.tile([128, 128], BF16, tag="pt", name="pt")
                                nc.tensor.transpose(
                                    pt, k_r[:, hh, half * HH:(half + 1) * HH], ident)
                                nc.vector.tensor_copy(
                                    kT[:, (c * hpk + hh) * 2 + half,
                                       st * 128:(st + 1) * 128], pt)
                for c in range(KCH):
                    vwc = []
                    for dt in range(d_tiles):
                        vwt = wstream.tile([128, kch_f], BF16, tag=f"w{dt}", bufs=2,
                                           name=f"vw{dt}")
                        nc.sync.dma_start(
                            out=vwt,
                            in_=vw_e[dt * 128:(dt + 1) * 128, c * kch_f:(c + 1) * kch_f])
                        vwc.append(vwt)
                    for st in range(sw_tiles):
                        pv = psA.tile([128, kch_f], F32, tag="pa1", bufs=2, name="pv")
                        for dt in range(d_tiles):
                            nc.tensor.matmul(pv, xT_sb[dt][:, st * 128:(st + 1) * 128],
                                             vwc[dt],
                                             start=(dt == 0), stop=(dt == d_tiles - 1))
                        nc.vector.tensor_copy(
                            v_sb[:, st, c * hpk:(c + 1) * hpk, 0:H],
                            pv.rearrange("p (h x) -> p h x", h=hpk))
                for st in range(sw_tiles):
                    nc.vector.memset(v_sb[:, st, :, H:H + 1], 1.0)

                # ---- Q projection (queries = last sq columns of the window) ----
                q0 = sw - sq
                for c in range(QCH):
                    qwc = []
                    for dt in range(d_tiles):
                        qwt = wstream.tile([128, qch_f], BF16, tag=f"w{dt}", bufs=2,
                                           name=f"qw{dt}")
                        nc.sync.dma_start(
                            out=qwt,
                            in_=qw_e[dt * 128:(dt + 1) * 128, c * qch_f:(c + 1) * qch_f])
                        qwc.append(qwt)
                    for qt in range(sq_tiles):
                        pq = psA.tile([128, qch_f], F32, tag="pa0", bufs=4, name="pq")
                        for dt in range(d_tiles):
                            nc.tensor.matmul(
                                pq, xT_sb[dt][:, q0 + qt * 128:q0 + (qt + 1) * 128],
                                qwc[dt],
                                start=(dt == 0), stop=(dt == d_tiles - 1))
                        q_n = scratch.tile([128, hpq, H], BF16, tag="x_n", bufs=3, name="q_n")
                        q_r = scratch.tile([128, hpq, H], BF16, tag="x_r", bufs=3, name="q_r")
                        norm_scale_copy(q_n, pq, hpq)
                        rope(q_r, q_n, tq_sb[:, qt, :, :], hpq)
                        for hh in range(hpq):
                            for half in range(2):
                                pt = psT.tile([128, 128], BF16, tag="pt", name="pt")
                                nc.tensor.transpose(
                                    pt, q_r[:, hh, half * HH:(half + 1) * HH], ident)
                                nc.vector.tensor_copy(
                                    qT[:, (c * hpq + hh) * 2 + half,
                                       qt * 128:(qt + 1) * 128], pt)

            # ---- Attention ----
            dch_f = min(512, d)
            DCH = d // dch_f
            with tc.tile_pool(name="attn", bufs=2) as attn:
                m01_sb = attn.tile([128, sw_tiles, sq], BF16, tag="m01", bufs=1)
                ow_first = []
                for i in range(nh * 2):
                    owt = attn.tile([128, dch_f], BF16, tag=f"ow{i}", bufs=2,
                                    name=f"ow{i}")
                    nc.sync.dma_start(out=owt, in_=ow_e[i * 128:(i + 1) * 128, 0:dch_f])
                    ow_first.append(owt)
                nc.sync.dma_start(out=m01_sb, in_=m01_e.rearrange("(t p) q -> p t q", p=128))
                WT = 1024 // 128  # window in 128-tiles
                off = sw_tiles - sq_tiles
                for n in range(nh):
                    khead = n // g
                    e_sb = attn.tile([128, sw_tiles, sq], BF16, tag="e", bufs=3, name="e_sb")
                    for r in range(sw_tiles):
                        # valid query tiles for this key tile (sliding window)
                        alo = max(0, r - off)
                        ahi = min(sq_tiles - 1, r - off + WT)
                        qlo, qhi = alo * 128, (ahi + 1) * 128
                        plg = psA.tile([128, sq], F32, tag="pa0", bufs=4, name="plg")
                        nc.tensor.matmul(plg[:, qlo:qhi],
                                         kT[:, khead * 2 + 0, r * 128:(r + 1) * 128],
                                         qT[:, n * 2 + 0, qlo:qhi], start=True, stop=False)
                        nc.tensor.matmul(plg[:, qlo:qhi],
                                         kT[:, khead * 2 + 1, r * 128:(r + 1) * 128],
                                         qT[:, n * 2 + 1, qlo:qhi], start=False, stop=True)
                        nc.scalar.activation(out=e_sb[:, r, qlo:qhi], in_=plg[:, qlo:qhi],
                                             func=mybir.ActivationFunctionType.Exp)
                        nc.vector.tensor_mul(e_sb[:, r, qlo:qhi], e_sb[:, r, qlo:qhi],
                                             m01_sb[:, r, qlo:qhi])
                    for qt in range(sq_tiles):
                        pe = psA.tile([128, H + 1], F32, tag="pa1", bufs=2, name="pe")
                        wq = off + qt
                        rvalid = range(max(0, wq - WT), wq + 1)
                        for ri, r in enumerate(rvalid):
                            nc.tensor.matmul(pe, e_sb[:, r, qt * 128:(qt + 1) * 128],
                                             v_sb[:, r, khead, :],
                                             start=(ri == 0), stop=(ri == len(rvalid) - 1))
                        rden = attn.tile([128, 1], F32, tag="rden", name="rden")
                        nc.vector.reciprocal(rden, pe[:, H:H + 1])
                        enc = attn.tile([128, H], BF16, tag="enc", name="enc")
                        nc.scalar.activation(out=enc, in_=pe[:, 0:H],
                                             func=mybir.ActivationFunctionType.Copy,
                                             scale=rden)
                        for half in range(2):
                            pt = psT.tile([128, 128], BF16, tag="pt", name="pt")
                            nc.tensor.transpose(pt, enc[:, half * HH:(half + 1) * HH], ident)
                            nc.vector.tensor_copy(encT[:, n * 2 + half, qt * 128:(qt + 1) * 128], pt)

                # ---- Output projection (same pool: ow chunk 0 prefetched above) ----
                for dc in range(DCH):
                    if dc == 0:
                        owc = ow_first
                    else:
                        owc = []
                        for i in range(nh * 2):
                            owt = attn.tile([128, dch_f], BF16, tag=f"ow{i}", bufs=2,
                                            name=f"ow{i}")
                            nc.sync.dma_start(
                                out=owt,
                                in_=ow_e[i * 128:(i + 1) * 128, dc * dch_f:(dc + 1) * dch_f])
                            owc.append(owt)
                    for qt in range(sq_tiles):
                        po = psA.tile([128, dch_f], F32, tag="pa0", bufs=4, name="po")
                        for i in range(nh * 2):
                            nc.tensor.matmul(po, encT[:, i, qt * 128:(qt + 1) * 128],
                                             owc[i],
                                             start=(i == 0), stop=(i == nh * 2 - 1))
                        o_sb = attn.tile([128, dch_f], F32, tag="o_sb", name="o_sb")
                        nc.vector.tensor_copy(o_sb, po)
                        nc.sync.dma_start(
                            out=out_e[qt * 128:(qt + 1) * 128, dc * dch_f:(dc + 1) * dch_f],
                            in_=o_sb)
    return nc


# ---------------------------------------------------------------------------
# Host side
# ---------------------------------------------------------------------------

def _rope_tables(pos, scale, extra=1.0):
    """Tables [L, 4, HH] = (C1, S1, C2, S2) with (1+scale) and `extra` folded."""
    frac = 2.0 * np.arange(HH, dtype=np.float64) / H
    ts = ROPE_BASE ** frac
    ang = pos[:, None].astype(np.float64) / ts[None, :]
    sin, cos = np.sin(ang), np.cos(ang)
    g1 = (1.0 + scale[:HH].astype(np.float64)) * extra   # gain on first half
    g2 = (1.0 + scale[HH:].astype(np.float64)) * extra   # gain on second half
    t = np.stack([cos * g1[None, :], sin * g2[None, :],
                  cos * g2[None, :], sin * g1[None, :]], axis=1)
    return t.astype(ml_dtypes.bfloat16)


_NC_CACHE = {}
_IN_MAPS_CACHE = {}


def _get_nc():
    if "nc" not in _NC_CACHE:
        nc = build_nc()
        nc.finalize()
        _NC_CACHE["nc"] = nc
    return _NC_CACHE["nc"]


def kernel(x, q_w, kv_w, q_scale, k_scale, out_w, positions, attn_mask):
    bf16 = ml_dtypes.bfloat16
    SQ, SW = 512, 1536
    n_chunk = T // SQ  # 4

    qw2 = np.ascontiguousarray(q_w.transpose(1, 0, 2).reshape(D, N * H)).astype(bf16)
    kw2 = np.ascontiguousarray(kv_w[0].transpose(1, 0, 2).reshape(D, K * H)).astype(bf16)
    vw2 = np.ascontiguousarray(kv_w[1].transpose(1, 0, 2).reshape(D, K * H)).astype(bf16)
    ow2 = np.ascontiguousarray(out_w.reshape(N * H, D)).astype(bf16)

    in_maps = []
    for c in range(8):
        b, j = divmod(c, 4)
        lo = (j + 1) * SQ - SW  # window start (may be negative -> zero pad)
        hi = (j + 1) * SQ
        xw = np.zeros((SW, D), np.float32)
        xw[max(0, -lo):] = x[b, max(lo, 0):hi]
        xT = np.ascontiguousarray(xw.T).astype(bf16)

        qpos = positions[b, j * SQ:(j + 1) * SQ]
        kpos = np.zeros((SW,), np.int32)
        kpos[max(0, -lo):] = positions[b, max(lo, 0):hi]
        tq = _rope_tables(qpos, q_scale, extra=H ** -0.5)
        tk = _rope_tables(kpos, k_scale)

        m = np.zeros((SQ, SW), np.float32)
        mvalid = attn_mask[b, 0, j * SQ:(j + 1) * SQ, max(lo, 0):hi]
        m[:, max(0, -lo):] = mvalid
        m01 = np.ascontiguousarray(m.T).astype(bf16)

        in_maps.append({"xT": xT, "qw": qw2, "kw": kw2, "vw": vw2, "ow": ow2,
                        "tq": tq, "tk": tk, "m01": m01})

    from concourse.bass_utils import run_bass_kernel_spmd
    _IN_MAPS_CACHE["in_maps"] = in_maps
    nc = _get_nc()
    res = run_bass_kernel_spmd(nc, in_maps, list(range(8)))
    out = np.empty((B, T, D), np.float32)
    for c in range(8):
        b, j = divmod(c, 4)
        out[b, j * SQ:(j + 1) * SQ] = res.results[c]["out"]
    return out



# revision 2
# speedup vs baseline: 1.3174x; 1.3174x over previous
"""Trainium2 Bass kernel for GQA sliding-window attention (nn_Attention_20375324852422).

Reference computation (B=2, T=2048, D=2560, N=8 q-heads, K=4 kv-heads, H=256,
WINDOW=1024):
    q = x @ q_w ; k,v = x @ kv_w      (GQA projections)
    q,k = rms_norm(q/k, scale)        (QK-norm, (1+scale) gain)
    q,k = rope(q/k, positions); q *= H**-0.5
    logits = q @ k.T  (grouped, sliding-window causal mask), softmax
    out = (probs @ v) @ out_w

Sharding: 8 cores = 2 (batch) x 4 (kv-heads).  Each core owns one batch row and
one kv head (plus its two grouped q heads) over the FULL sequence, so no
projection work is replicated anywhere (the old seq-chunk layout recomputed K/V
3x).  Each core emits a partial output (its 2 heads' contribution through
out_w); the host sums the 4 partials per batch row - no collectives.

Per-core device layouts (host prepares, see kernel()):
    xT   [D, 2048]   x[b] transposed (bf16)       - lhsT for projections
    kvw  [D, 512]    [kw | vw] for the kv head    - fused rhs (one N=512 matmul
    qw   [D, 512]    2 q heads                      chain per seq tile)
    ow   [512, D]    out_w rows for the 2 heads
    tq   [2048, 4, 128] / tk [...]                - RoPE tables (C1,S1,C2,S2)
                     with (1+scale) gains and (for q) H**-0.5 folded in
    tri  [2, 128, 128] {0,1} triangle masks: [0]=diag tile (key<=query),
                     [1]=far tile (key>query)  -- the sliding-window mask is
                     canonical (host-checked; numpy fallback otherwise)
    out  [2048, D]   fp32 partial
"""

import numpy as np
import ml_dtypes

import concourse.bass as bass
import concourse.tile as tile
from concourse import bacc
from concourse import mybir
from concourse.masks import make_identity

BF16 = mybir.dt.bfloat16
F32 = mybir.dt.float32

B, T, D, N, K, H = 2, 2048, 2560, 8, 4, 256
G = N // K
WINDOW = 1024
ROPE_BASE = 10000
EPS = 1e-6
HH = H // 2  # 128

D_TILES = D // 128   # 20
S_TILES = T // 128   # 16
NCH = T // 512       # 4 query chunks of 512
WT = WINDOW // 128   # 8


def build_nc():
    """Per-core Bass graph: full-T GQA attention for 2 q-heads / 1 kv-head."""
    nc = bacc.Bacc()
    nh = 2

    xT_e = nc.dram_tensor("xT", [D, T], BF16, kind="ExternalInput")
    kvw_e = nc.dram_tensor("kvw", [D, 2 * H], BF16, kind="ExternalInput")
    qw_e = nc.dram_tensor("qw", [D, nh * H], BF16, kind="ExternalInput")
    ow_e = nc.dram_tensor("ow", [nh * H, D], BF16, kind="ExternalInput")
    tq_e = nc.dram_tensor("tq", [T, 4, HH], BF16, kind="ExternalInput")
    tk_e = nc.dram_tensor("tk", [T, 4, HH], BF16, kind="ExternalInput")
    tri_e = nc.dram_tensor("tri", [2, 128, 128], BF16, kind="ExternalInput")
    out_e = nc.dram_tensor("out", [T, D], F32, kind="ExternalOutput")

    with tile.TileContext(nc) as tc:
        with (
            tc.tile_pool(name="const", bufs=1) as const,
            tc.tile_pool(name="persist", bufs=1) as persist,
            tc.tile_pool(name="psA", bufs=1, space="PSUM") as psA,
            tc.tile_pool(name="psT", bufs=2, space="PSUM") as psT,
        ):
            ident = const.tile([128, 128], BF16)
            make_identity(nc, ident)
            eps_t = const.tile([128, 1], F32)
            nc.vector.memset(eps_t, EPS)
            tri_sb = const.tile([128, 2, 128], BF16)
            nc.sync.dma_start(out=tri_sb, in_=tri_e.rearrange("t p q -> p t q"))

            kT = persist.tile([128, 2, T], BF16)          # [h, kv half, s]
            v_sb = persist.tile([128, S_TILES, H + 1], BF16)
            qT = persist.tile([128, nh * 2, T], BF16)     # [h, head*2+half, s]
            ow_sb = persist.tile([128, nh * 2, D], BF16)  # [nh-row-tile, d]
            nc.sync.dma_start(out=ow_sb, in_=ow_e.rearrange("(i p) d -> p i d", p=128))

            def rope(dst, src, tbl, heads):
                """dst/src: [128, heads, H] sbuf; tbl: [128, 4, HH] slice."""
                first = src[:, :, 0:HH]
                second = src[:, :, HH:H]
                c1 = tbl[:, 0, :].unsqueeze(1).broadcast_to([128, heads, HH])
                s1 = tbl[:, 1, :].unsqueeze(1).broadcast_to([128, heads, HH])
                c2 = tbl[:, 2, :].unsqueeze(1).broadcast_to([128, heads, HH])
                s2 = tbl[:, 3, :].unsqueeze(1).broadcast_to([128, heads, HH])
                t1 = scratch.tile([128, heads, HH], F32, tag="rp1", name="t1")
                t2 = scratch.tile([128, heads, HH], F32, tag="rp2", name="t2")
                nc.vector.tensor_mul(t1, first, c1)
                nc.vector.tensor_mul(t2, second, s1)
                nc.vector.tensor_sub(dst[:, :, 0:HH], t1, t2)
                nc.vector.tensor_mul(t1, second, c2)
                nc.vector.tensor_mul(t2, first, s2)
                nc.vector.tensor_add(dst[:, :, HH:H], t1, t2)

            def norm_scale_copy(dst, psrc, heads):
                """RMS-normalize psum [128, heads*H] into sbuf dst [128, heads, H]."""
                ssq = scratch.tile([128, heads], F32, tag="ssq", name="ssq")
                sq_junk = scratch.tile([128, H], BF16, tag="sqj", name="sqj")
                for hh in range(heads):
                    nc.scalar.activation(
                        out=sq_junk, in_=psrc[:, hh * H:(hh + 1) * H],
                        func=mybir.ActivationFunctionType.Square,
                        accum_out=ssq[:, hh:hh + 1])
                nc.scalar.activation(
                    out=ssq, in_=ssq, func=mybir.ActivationFunctionType.Sqrt,
                    bias=eps_t, scale=1.0 / H)
                nc.vector.reciprocal(ssq, ssq)
                for hh in range(heads):
                    nc.scalar.activation(
                        out=dst[:, hh, :], in_=psrc[:, hh * H:(hh + 1) * H],
                        func=mybir.ActivationFunctionType.Copy,
                        scale=ssq[:, hh:hh + 1])

            with (
                tc.tile_pool(name="xpool", bufs=1) as xpool,
                tc.tile_pool(name="wstream", bufs=2) as wstream,
                tc.tile_pool(name="tstream", bufs=3) as tstream,
                tc.tile_pool(name="scratch", bufs=2) as scratch,
            ):
                xT_sb = []
                for dt in range(D_TILES):
                    xt = xpool.tile([128, T], BF16, tag=f"x{dt}", name=f"x{dt}")
                    xT_sb.append(xt)

                # ---- K/V projection (fused [kw|vw] rhs, N=512) ----
                kvc = []
                for dt in range(D_TILES):
                    kvt = wstream.tile([128, 2 * H], BF16, tag=f"w{dt}", bufs=2,
                                       name=f"kvw{dt}")
                    nc.sync.dma_start(
                        out=kvt, in_=kvw_e[dt * 128:(dt + 1) * 128, :])
                    kvc.append(kvt)
                    # interleave the first x chunk with the weights so the
                    # first matmul group isn't queued behind the full 10MB xT
                    nc.sync.dma_start(
                        out=xT_sb[dt][:, 0:512],
                        in_=xT_e[dt * 128:(dt + 1) * 128, 0:512])
                for sc in range(1, 4):
                    for dt in range(D_TILES):
                        nc.sync.dma_start(
                            out=xT_sb[dt][:, sc * 512:(sc + 1) * 512],
                            in_=xT_e[dt * 128:(dt + 1) * 128, sc * 512:(sc + 1) * 512])

                for st in range(S_TILES):
                    tkt = tstream.tile([128, 4, HH], BF16, tag="tbl", name="tkt")
                    nc.sync.dma_start(out=tkt, in_=tk_e[st * 128:(st + 1) * 128])
                    pkv = psA.tile([128, 2 * H], F32, tag="pa0", bufs=4, name="pkv")
                    for dt in range(D_TILES):
                        nc.tensor.matmul(pkv, xT_sb[dt][:, st * 128:(st + 1) * 128],
                                         kvc[dt],
                                         start=(dt == 0), stop=(dt == D_TILES - 1))
                    k_n = scratch.tile([128, 1, H], BF16, tag="x_n", bufs=3, name="k_n")
                    k_r = scratch.tile([128, 1, H], BF16, tag="x_r", bufs=3, name="k_r")
                    norm_scale_copy(k_n, pkv[:, 0:H], 1)
                    rope(k_r, k_n, tkt, 1)
                    for half in range(2):
                        pt = psT.tile([128, 128], BF16, tag="pt", name="pt")
                        nc.tensor.transpose(
                            pt, k_r[:, 0, half * HH:(half + 1) * HH], ident)
                        nc.vector.tensor_copy(
                            kT[:, half, st * 128:(st + 1) * 128], pt)
                    nc.vector.tensor_copy(v_sb[:, st, 0:H], pkv[:, H:2 * H])
                    nc.vector.memset(v_sb[:, st, H:H + 1], 1.0)

                # ---- Q projection ----
                for st in range(S_TILES):
                    if st == 0:
                        qwc = []
                        for dt in range(D_TILES):
                            qwt = wstream.tile([128, nh * H], BF16, tag=f"w{dt}",
                                               bufs=2, name=f"qw{dt}")
                            nc.sync.dma_start(
                                out=qwt, in_=qw_e[dt * 128:(dt + 1) * 128, :])
                            qwc.append(qwt)
                    tqt = tstream.tile([128, 4, HH], BF16, tag="tbl", name="tqt")
                    nc.sync.dma_start(out=tqt, in_=tq_e[st * 128:(st + 1) * 128])
                    pq = psA.tile([128, nh * H], F32, tag="pa0", bufs=4, name="pq")
                    for dt in range(D_TILES):
                        nc.tensor.matmul(pq, xT_sb[dt][:, st * 128:(st + 1) * 128],
                                         qwc[dt],
                                         start=(dt == 0), stop=(dt == D_TILES - 1))
                    q_n = scratch.tile([128, nh, H], BF16, tag="q_n", bufs=3, name="q_n")
                    q_r = scratch.tile([128, nh, H], BF16, tag="q_r", bufs=3, name="q_r")
                    norm_scale_copy(q_n, pq, nh)
                    rope(q_r, q_n, tqt, nh)
                    for hh in range(nh):
                        for half in range(2):
                            pt = psT.tile([128, 128], BF16, tag="pt", name="pt")
                            nc.tensor.transpose(
                                pt, q_r[:, hh, half * HH:(half + 1) * HH], ident)
                            nc.vector.tensor_copy(
                                qT[:, hh * 2 + half, st * 128:(st + 1) * 128], pt)

            # ---- Attention + output projection, per 512-query chunk ----
            with tc.tile_pool(name="attn", bufs=2) as attn:
                for c in range(NCH):
                    r0 = 4 * c - WT          # first key tile slot (may be <0)
                    R = range(max(0, r0), 4 * c + 4)
                    encT = attn.tile([128, nh * 2, 512], BF16, tag="encT",
                                     bufs=2, name="encT")
                    for n in range(nh):
                        e_sb = attn.tile([128, WT + 4, 512], BF16, tag="e",
                                         bufs=3, name="e_sb")
                        for r in R:
                            t = r - r0
                            lo = max(0, r - 4 * c)
                            hi = min(3, r + WT - 4 * c)
                            qlo, qhi = lo * 128, (hi + 1) * 128
                            plg = psA.tile([128, 512], F32, tag="pa0", bufs=4,
                                           name="plg")
                            nc.tensor.matmul(
                                plg[:, qlo:qhi],
                                kT[:, 0, r * 128:(r + 1) * 128],
                                qT[:, n * 2 + 0, c * 512 + qlo:c * 512 + qhi],
                                start=True, stop=False)
                            nc.tensor.matmul(
                                plg[:, qlo:qhi],
                                kT[:, 1, r * 128:(r + 1) * 128],
                                qT[:, n * 2 + 1, c * 512 + qlo:c * 512 + qhi],
                                start=False, stop=True)
                            nc.scalar.activation(
                                out=e_sb[:, t, qlo:qhi], in_=plg[:, qlo:qhi],
                                func=mybir.ActivationFunctionType.Exp)
                            if 4 * c <= r <= 4 * c + 3:      # diagonal tile
                                dq = r - 4 * c
                                nc.vector.tensor_mul(
                                    e_sb[:, t, dq * 128:(dq + 1) * 128],
                                    e_sb[:, t, dq * 128:(dq + 1) * 128],
                                    tri_sb[:, 0, :])
                            if 4 * c <= r + WT <= 4 * c + 3:  # far (window edge)
                                df = r + WT - 4 * c
                                nc.vector.tensor_mul(
                                    e_sb[:, t, df * 128:(df + 1) * 128],
                                    e_sb[:, t, df * 128:(df + 1) * 128],
                                    tri_sb[:, 1, :])
                        for qt in range(4):
                            qg = 4 * c + qt
                            rvalid = range(max(0, qg - WT), qg + 1)
                            pe = psA.tile([128, H + 1], F32, tag="pa1", bufs=2,
                                          name="pe")
                            for ri, r in enumerate(rvalid):
                                nc.tensor.matmul(
                                    pe, e_sb[:, r - r0, qt * 128:(qt + 1) * 128],
                                    v_sb[:, r, :],
                                    start=(ri == 0), stop=(ri == len(rvalid) - 1))
                            rden = attn.tile([128, 1], F32, tag="rden", name="rden")
                            nc.vector.reciprocal(rden, pe[:, H:H + 1])
                            enc = attn.tile([128, H], BF16, tag="enc", name="enc")
                            nc.scalar.activation(
                                out=enc, in_=pe[:, 0:H],
                                func=mybir.ActivationFunctionType.Copy, scale=rden)
                            for half in range(2):
                                pt = psT.tile([128, 128], BF16, tag="pt", name="pt")
                                nc.tensor.transpose(
                                    pt, enc[:, half * HH:(half + 1) * HH], ident)
                                nc.vector.tensor_copy(
                                    encT[:, n * 2 + half, qt * 128:(qt + 1) * 128],
                                    pt)
                    for dc in range(D // 512):
                        for qt in range(4):
                            po = psA.tile([128, 512], F32, tag="pa0", bufs=4,
                                          name="po")
                            for i in range(nh * 2):
                                nc.tensor.matmul(
                                    po, encT[:, i, qt * 128:(qt + 1) * 128],
                                    ow_sb[:, i, dc * 512:(dc + 1) * 512],
                                    start=(i == 0), stop=(i == nh * 2 - 1))
                            o_sb = attn.tile([128, 512], F32, tag="o_sb", bufs=3,
                                             name="o_sb")
                            nc.vector.tensor_copy(o_sb, po)
                            nc.sync.dma_start(
                                out=out_e[(4 * c + qt) * 128:(4 * c + qt + 1) * 128,
                                          dc * 512:(dc + 1) * 512],
                                in_=o_sb)
    return nc


# ---------------------------------------------------------------------------
# Host side
# ---------------------------------------------------------------------------

def _rope_tables(pos, scale, extra=1.0):
    """Tables [L, 4, HH] = (C1, S1, C2, S2) with (1+scale) and `extra` folded."""
    frac = 2.0 * np.arange(HH, dtype=np.float64) / H
    ts = ROPE_BASE ** frac
    ang = pos[:, None].astype(np.float64) / ts[None, :]
    sin, cos = np.sin(ang), np.cos(ang)
    g1 = (1.0 + scale[:HH].astype(np.float64)) * extra   # gain on first half
    g2 = (1.0 + scale[HH:].astype(np.float64)) * extra   # gain on second half
    t = np.stack([cos * g1[None, :], sin * g2[None, :],
                  cos * g2[None, :], sin * g1[None, :]], axis=1)
    return t.astype(ml_dtypes.bfloat16)


def _canonical_mask():
    qp = np.arange(T)[:, None]
    kp = np.arange(T)[None, :]
    return (kp <= qp) & ((qp - kp) < WINDOW)


def _numpy_reference(x, q_w, kv_w, q_scale, k_scale, out_w, positions, attn_mask):
    """Slow numpy fallback (only used if attn_mask isn't the canonical
    sliding-window pattern, which never happens for this problem's inputs)."""
    def rms(v, s):
        var = np.mean(np.square(v), axis=-1, keepdims=True)
        return v / np.sqrt(var + EPS) * (1.0 + s)

    def rope_np(v, pos):
        hd = v.shape[-1]
        ts = ROPE_BASE ** (2 * np.arange(hd // 2) / hd)
        ang = pos[..., None] / ts
        ang = ang[..., None, :]
        s, c = np.sin(ang), np.cos(ang)
        f, sec = v[..., :hd // 2], v[..., hd // 2:]
        return np.concatenate([f * c - sec * s, sec * c + f * s], -1)

    q = np.einsum('BTD,NDH->BTNH', x, q_w)
    k = np.einsum('BSD,KDH->BSKH', x, kv_w[0])
    v = np.einsum('BSD,KDH->BSKH', x, kv_w[1])
    q = rope_np(rms(q, q_scale), positions) * H ** -0.5
    k = rope_np(rms(k, k_scale), positions)
    qg = q.reshape(B, T, K, G, H)
    logits = np.einsum('BTKGH,BSKH->BTKGS', qg, k).reshape(B, T, N, T)
    bmask = attn_mask[:, 0][:, :, None, :]
    masked = np.where(bmask, logits, -2.3819763e+38)
    m = masked.max(-1, keepdims=True)
    p = np.exp(masked - m)
    p /= p.sum(-1, keepdims=True)
    enc = np.einsum('BTKGS,BSKH->BTKGH', p.reshape(B, T, K, G, T), v)
    return np.einsum('BTNH,NHD->BTD', enc.reshape(B, T, N, H), out_w)


_NC_CACHE = {}
_IN_MAPS_CACHE = {}


def _get_nc():
    if "nc" not in _NC_CACHE:
        nc = build_nc()
        nc.finalize()
        _NC_CACHE["nc"] = nc
    return _NC_CACHE["nc"]


def kernel(x, q_w, kv_w, q_scale, k_scale, out_w, positions, attn_mask):
    bf16 = ml_dtypes.bfloat16

    if not np.array_equal(
            np.asarray(attn_mask),
            np.broadcast_to(_canonical_mask()[None, None], (B, 1, T, T))):
        return _numpy_reference(
            np.asarray(x, np.float32), np.asarray(q_w, np.float32),
            np.asarray(kv_w, np.float32), np.asarray(q_scale, np.float32),
            np.asarray(k_scale, np.float32), np.asarray(out_w, np.float32),
            np.asarray(positions), np.asarray(attn_mask)).astype(np.float32)

    tri = np.zeros((2, 128, 128), np.float32)
    tri[0] = np.triu(np.ones((128, 128)))        # diag tile: key <= query
    tri[1] = np.tril(np.ones((128, 128)), -1)    # far tile:  key > query
    tri = tri.astype(bf16)

    in_maps = []
    for c in range(8):
        b, j = divmod(c, 4)
        xT = np.ascontiguousarray(np.asarray(x[b]).T).astype(bf16)
        kvw = np.ascontiguousarray(
            np.concatenate([kv_w[0, j], kv_w[1, j]], axis=1)).astype(bf16)
        qw = np.ascontiguousarray(
            q_w[2 * j:2 * j + 2].transpose(1, 0, 2).reshape(D, 2 * H)).astype(bf16)
        ow = np.ascontiguousarray(
            out_w[2 * j:2 * j + 2].reshape(2 * H, D)).astype(bf16)
        pos = np.asarray(positions[b])
        tq = _rope_tables(pos, np.asarray(q_scale), extra=H ** -0.5)
        tk = _rope_tables(pos, np.asarray(k_scale))
        in_maps.append({"xT": xT, "kvw": kvw, "qw": qw, "ow": ow,
                        "tq": tq, "tk": tk, "tri": tri})

    from concourse.bass_utils import run_bass_kernel_spmd
    _IN_MAPS_CACHE["in_maps"] = in_maps
    nc = _get_nc()
    res = run_bass_kernel_spmd(nc, in_maps, list(range(8)))
    out = np.empty((B, T, D), np.float32)
    for b in range(B):
        acc = res.results[4 * b]["out"].astype(np.float32)
        for j in range(1, 4):
            acc = acc + res.results[4 * b + j]["out"]
        out[b] = acc
    return out
